# revision 11
# baseline (speedup 1.0000x reference)
"""GIN (graph isomorphism network) message passing on 8 Trainium2 NeuronCores.

Distribution: nodes (dst) sharded contiguously across the 8 cores; each core
owns all edges incident to its dst shard. Per layer:

  * 4 gather passes per core stream y[src] rows out of HBM into SBUF with
    multi-queue dma_gather (edge-major staging, dst-sorted segment-aligned
    chunks of 256 slots).
  * Each chunk is reduced to <=32 per-dst partial sums by a PE matmul whose
    stationary operand is a [128,32] one-hot membership matrix built on the
    Vector engine from per-slot segment ids (edge weights folded in), so the
    PSUM column layout is input-independent (one SPMD program for all cores).
  * Partials are dma_scatter_add-ed into a zeroed DRAM accumulator. Chunks
    are segment-aligned, so within a pass every dst has at most one partial
    (no read-modify-write races); pad partials land on a trash row.
  * Layer 1 aggregates raw x (512B rows) and applies W1a afterwards, using
    (A@x)@W = A@(x@W) associativity, so no h-table exists for layer 1 and x
    is simply replicated per core (no communication).
  * For layers 2..3 the h-table [100000,64] is rebuilt after each layer by 4
    sliced AllGathers (25000-row windows also keep dma_gather's int16 index
    range happy).
  * MLP + BatchNorm run feature-major (weights stationary on PE); BN stats
    are summed with a tiny AllReduce. Residual adds per reference semantics.

Everything is fp32; the only deviations from the jax reference are benign
reassociations (fp32 accumulate order), measured at ~1e-5 relative error.
"""

import sys

sys.path.insert(0, '/opt/trn_rl_repo')

import numpy as np

LAST_EXEC_NS = None
N_CORES = 8
N = 100000
F = 128
H = 64
SHARD = N // N_CORES        # 12500
SLICE = SHARD // 4          # 3125 rows per core per table AllGather
WIN1 = N // 4               # 25000-row gather window (layer 1 / x)
WIN2 = 2 * SHARD            # 25000-row table window (layers 2..3)
AGG_ROWS = 12800            # DRAM accumulator rows; [12500, 12800) unused/trash
TRASH = 12544
CHUNK = 256                 # edge slots per chunk (2 staging columns)
MAXSEG = 32                 # partial slots per chunk
GCALL = 1024                # slots per dma_gather / dma_scatter_add call
BN_EPS = 1e-5
NGRP = (SHARD + 511) // 512  # 25 feature-major column groups (24*512 + 212)


# --------------------------------------------------------------------------
# host-side edge preprocessing
# --------------------------------------------------------------------------

def _build_chunks(order, dst_local, pass_ids, npass):
    """Pack each pass's dst-sorted edges into segment-aligned chunks.

    Returns [npass][nchunk] of (slot_edge_ids, seg_dsts). A dst's edges never
    span chunks, chunks hold <= CHUNK slots and <= MAXSEG segments.
    """
    out = [[] for _ in range(npass)]
    pass_of_sorted = pass_ids[order]
    for p in range(npass):
        eids = order[pass_of_sorted == p]
        if eids.size == 0:
            continue
        dl = dst_local[eids]
        bnd = np.flatnonzero(np.diff(dl)) + 1
        seg_starts = np.concatenate(([0], bnd))
        seg_ends = np.concatenate((bnd, [dl.size]))
        cur_s, cur_g = [], []
        for s0, s1 in zip(seg_starts, seg_ends):
            if s1 - s0 > CHUNK:
                raise AssertionError(
                    f"in-degree within one pass exceeds {CHUNK}; unsupported")
            if cur_g and (len(cur_s) + (s1 - s0) > CHUNK or
                          len(cur_g) + 1 > MAXSEG):
                out[p].append((np.asarray(cur_s, np.int64),
                               np.asarray(cur_g, np.int64)))
                cur_s, cur_g = [], []
            cur_s.extend(eids[s0:s1].tolist())
            cur_g.append(int(dl[s0]))
        if cur_g:
            out[p].append((np.asarray(cur_s, np.int64),
                           np.asarray(cur_g, np.int64)))
    return out


def _pack_meta(chunks_pp, nch_pad, gather_idx, ew, dst_local):
    """Pack chunks into the flat device metadata arrays."""
    tot_chunks = int(np.sum(nch_pad))
    tot_slots = CHUNK * tot_chunks
    gidx_flat = np.zeros(tot_slots, np.int16)
    segid = np.full((CHUNK, tot_chunks), -1.0, np.float32)
    wcol = np.zeros((CHUNK, tot_chunks), np.float32)
    sidx_flat = np.full(MAXSEG * tot_chunks, TRASH, np.int16)
    ci = 0
    for p, nch in enumerate(nch_pad):
        for k in range(nch):
            if k < len(chunks_pp[p]):
                slots, segs = chunks_pp[p][k]
                nsl = slots.size
                gidx_flat[ci * CHUNK:ci * CHUNK + nsl] = gather_idx[slots]
                dl = dst_local[slots]
                seg_map = {int(d): j for j, d in enumerate(segs.tolist())}
                segid[:nsl, ci] = [seg_map[int(d)] for d in dl]
                wcol[:nsl, ci] = ew[slots]
                sidx_flat[ci * MAXSEG:ci * MAXSEG + segs.size] = segs
            ci += 1
    # gather idx stream: within each GCALL-slot call, idx i sits at
    # partition i%16, free col i//16, replicated over the 8 groups of 16.
    gidx = np.zeros((128, tot_slots // 16), np.int16)
    for c in range(tot_slots // GCALL):
        blk = gidx_flat[c * GCALL:(c + 1) * GCALL].reshape(GCALL // 16, 16).T
        gidx[:, c * (GCALL // 16):(c + 1) * (GCALL // 16)] = np.tile(blk, (8, 1))
    # per-slot metadata in staging-column layout: slot i of chunk ci sits at
    # partition i%128, column 2*ci + i//128.
    segid2 = np.empty((128, 2 * tot_chunks), np.float32)
    wcol2 = np.empty((128, 2 * tot_chunks), np.float32)
    segid2[:, 0::2] = segid[:128, :]
    segid2[:, 1::2] = segid[128:, :]
    wcol2[:, 0::2] = wcol[:128, :]
    wcol2[:, 1::2] = wcol[128:, :]
    # scatter idx stream (partial slots), same 16-wrap layout per 1024-call
    tot_ps = MAXSEG * tot_chunks
    sidx = np.zeros((128, tot_ps // 16), np.int16)
    for c in range(tot_ps // GCALL):
        blk = sidx_flat[c * GCALL:(c + 1) * GCALL].reshape(GCALL // 16, 16).T
        sidx[:, c * (GCALL // 16):(c + 1) * (GCALL // 16)] = np.tile(blk, (8, 1))
    return dict(gidx=gidx, segid=segid2, wcol=wcol2, sidx=sidx)


def _preprocess(edge_index, edge_weight):
    src = edge_index[0].astype(np.int64)
    dst = edge_index[1].astype(np.int64)
    ew = edge_weight.astype(np.float32)
    cores = []
    for c in range(N_CORES):
        lo = SHARD * c
        sel = np.flatnonzero((dst >= lo) & (dst < lo + SHARD))
        dsub = dst[sel] - lo
        ssub = src[sel]
        wsub = ew[sel]
        # layer 1: x rolled by -SHARD*c per core, windows of WIN1
        rs = (ssub - lo) % N
        p1 = rs // WIN1
        i1 = rs % WIN1
        o1 = np.lexsort((ssub, dsub, p1))
        # layers 2..3: table slice j holds rows c'*SLICE + (src%SHARD - SLICE*j)
        p2 = (ssub % SHARD) // SLICE
        i2 = (ssub // SHARD) * SLICE + (ssub % SHARD) - p2 * SLICE
        o2 = np.lexsort((ssub, dsub, p2))
        cores.append((_build_chunks(o1, dsub, p1, 4), i1,
                      _build_chunks(o2, dsub, p2, 4), i2, dsub, wsub))
    # uniform per-pass chunk counts (multiple of 32 -> whole scatter groups)
    nch1 = [max(32, -(-max(len(cc[0][p]) for cc in cores) // 32) * 32)
            for p in range(4)]
    nch2 = [max(32, -(-max(len(cc[2][p]) for cc in cores) // 32) * 32)
            for p in range(4)]
    metas = []
    for ch1, i1, ch2, i2, dsub, wsub in cores:
        metas.append((_pack_meta(ch1, nch1, i1, wsub, dsub),
                      _pack_meta(ch2, nch2, i2, wsub, dsub)))
    return metas, nch1, nch2


# --------------------------------------------------------------------------
# device program
# --------------------------------------------------------------------------

def _build_program(nch1, nch2, eps1, epss):
    import concourse.bass as bass  # noqa: F401
    import concourse.tile as tile
    from concourse import bacc, mybir
    from concourse.masks import make_identity

    tot1 = int(np.sum(nch1))
    tot2 = int(np.sum(nch2))

    nc = bacc.Bacc("TRN2", target_bir_lowering=False, debug=False,
                   num_devices=N_CORES, num_swdge_queues=4)

    dt = mybir.dt
    Alu = mybir.AluOpType
    Act = mybir.ActivationFunctionType
    groups = [list(range(N_CORES))]

    x_in = nc.dram_tensor("x_in", [N, F], dt.float32, kind="ExternalInput")
    gidx1_in = nc.dram_tensor("gidx1", [128, tot1 * CHUNK // 16], dt.int16, kind="ExternalInput")
    seg1_in = nc.dram_tensor("seg1", [128, 2 * tot1], dt.float32, kind="ExternalInput")
    w1_in = nc.dram_tensor("w1", [128, 2 * tot1], dt.float32, kind="ExternalInput")
    sidx1_in = nc.dram_tensor("sidx1", [128, tot1 * MAXSEG // 16], dt.int16, kind="ExternalInput")
    gidx2_in = nc.dram_tensor("gidx2", [128, tot2 * CHUNK // 16], dt.int16, kind="ExternalInput")
    seg2_in = nc.dram_tensor("seg2", [128, 2 * tot2], dt.float32, kind="ExternalInput")
    w2_in = nc.dram_tensor("w2", [128, 2 * tot2], dt.float32, kind="ExternalInput")
    sidx2_in = nc.dram_tensor("sidx2", [128, tot2 * MAXSEG // 16], dt.int16, kind="ExternalInput")
    iota_in = nc.dram_tensor("iota", [128, MAXSEG], dt.float32, kind="ExternalInput")
    w1a_in = nc.dram_tensor("w1a", [F, H], dt.float32, kind="ExternalInput")
    w1b_in = nc.dram_tensor("w1b", [H, H], dt.float32, kind="ExternalInput")
    wsa_in = nc.dram_tensor("wsa", [2, H, H], dt.float32, kind="ExternalInput")
    wsb_in = nc.dram_tensor("wsb", [2, H, H], dt.float32, kind="ExternalInput")
    # vecs columns: 0 b1a, 1 g1, 2 be1, 3 bsa0, 4 gs0, 5 bes0, 6 bsa1,
    # 7 gs1, 8 bes1, 9 b1b, 10 bsb0, 11 bsb1, 12 eps1+1, 13 epss0+1, 14 epss1+1
    vecs_in = nc.dram_tensor("vecs", [64, 16], dt.float32, kind="ExternalInput")
    out_t = nc.dram_tensor("out", [SHARD, H], dt.float32, kind="ExternalOutput")

    with tile.TileContext(nc) as tc:
        with (
            tc.tile_pool(name="meta", bufs=1) as mpool,
            tc.tile_pool(name="passmeta", bufs=2) as ppool,
            tc.tile_pool(name="stage", bufs=6) as spool,
            tc.tile_pool(name="small", bufs=8) as qpool,
            tc.tile_pool(name="slab", bufs=3) as bpool,
            tc.tile_pool(name="psum", bufs=2, space="PSUM") as psum_pool,
            tc.tile_pool(name="dram", bufs=1, space="DRAM") as dram_pool,
        ):
            # ---------------- persistent small tiles ----------------
            iota_sb = mpool.tile([128, MAXSEG], dt.float32)
            nc.sync.dma_start(iota_sb[:], iota_in[:])
            w1a_sb = mpool.tile([F, H], dt.float32)
            w1b_sb = mpool.tile([H, H], dt.float32)
            wsa_sb = mpool.tile([H, 2 * H], dt.float32)
            wsb_sb = mpool.tile([H, 2 * H], dt.float32)
            vecs_sb = mpool.tile([64, 16], dt.float32)
            nc.sync.dma_start(w1a_sb[:], w1a_in[:])
            nc.sync.dma_start(w1b_sb[:], w1b_in[:])
            for l in range(2):
                nc.sync.dma_start(wsa_sb[:, 64 * l:64 * (l + 1)], wsa_in[l])
                nc.sync.dma_start(wsb_sb[:, 64 * l:64 * (l + 1)], wsb_in[l])
            nc.sync.dma_start(vecs_sb[:], vecs_in[:])
            ident = mpool.tile([128, 128], dt.float32)
            make_identity(nc, ident[:])
            zero512 = mpool.tile([128, 512], dt.float32)
            nc.gpsimd.memset(zero512[:], 0.0)
            epsb = mpool.tile([64, 1], dt.float32)
            nc.gpsimd.memset(epsb[:], BN_EPS)
            sidx1_sb = mpool.tile([128, tot1 * MAXSEG // 16], dt.int16)
            sidx2_sb = mpool.tile([128, tot2 * MAXSEG // 16], dt.int16)
            nc.sync.dma_start(sidx1_sb[:], sidx1_in[:])
            nc.sync.dma_start(sidx2_sb[:], sidx2_in[:])

            # ---------------- dram scratch ----------------
            agg = []
            for li, ft in ((0, F), (1, H), (2, H)):
                agg_l = dram_pool.tile([AGG_ROWS, ft], dt.float32, tag=f"agg{li}")
                agg.append(agg_l)
            uT_dram = dram_pool.tile([F, SHARD], dt.float32)
            z2T_dram = dram_pool.tile([H, SHARD], dt.float32)
            hT_dram = []
            for li in range(3):
                hT_l = dram_pool.tile([H, SHARD], dt.float32, tag=f"hT{li}")
                hT_dram.append(hT_l)
            shard_nm = []
            for li in range(2):
                nm_l = dram_pool.tile([SHARD, H], dt.float32, tag=f"nm{li}")
                shard_nm.append(nm_l)
            tables = [[], []]
            for l in range(2):
                for j in range(4):
                    tbl_lj = dram_pool.tile([WIN2, H], dt.float32, tag=f"tbl{l}{j}")
                    tables[l].append(tbl_lj)
            red_io = dram_pool.tile([64, 2], dt.float32)
            red_oo = dram_pool.tile([64, 2], dt.float32, tag="red_oo")
            warm_i = dram_pool.tile([64, 2], dt.float32, tag="warm_i")
            warm_o = dram_pool.tile([64, 2], dt.float32, tag="warm_o")

            # warm up the collective stack early (first collective has
            # ~130us of one-time setup; overlap it with layer-1 gather)
            wt = qpool.tile([64, 2], dt.float32, tag="warm")
            nc.gpsimd.memset(wt[:], 0.0)
            nc.sync.dma_start(warm_i[:], wt[:])
            nc.gpsimd.collective_compute(
                "AllReduce", Alu.add, replica_groups=groups,
                ins=[warm_i.opt()], outs=[warm_o.opt()])

            # ---------------- aggregation ----------------
            pool_dma_counter = [0]

            def emit_aggregation(nch, feat, table_aps, gidx_in_, seg_in_,
                                 w_in_, sidx_sb, agg_dram):
                # zero the accumulator
                total_free = AGG_ROWS * feat // 128
                aggz = agg_dram[:].rearrange("a b -> (a b)").rearrange(
                    "(p f) -> p f", p=128)
                for z0 in range(0, total_free, 512):
                    z1 = min(z0 + 512, total_free)
                    nc.sync.dma_start(aggz[:, z0:z1], zero512[:, :z1 - z0])
                chunk_base = 0
                for p in range(4):
                    ncalls = nch[p] * CHUNK // GCALL      # multiple of 8
                    # per-pass metadata
                    gix = ppool.tile([128, nch[p] * CHUNK // 16], dt.int16,
                                     tag="gix")
                    sgx = ppool.tile([128, 2 * nch[p]], dt.float32, tag="sgx")
                    wgx = ppool.tile([128, 2 * nch[p]], dt.float32, tag="wgx")
                    nc.sync.dma_start(
                        gix[:], gidx_in_[:, chunk_base * CHUNK // 16:
                                         (chunk_base + nch[p]) * CHUNK // 16])
                    nc.sync.dma_start(
                        sgx[:], seg_in_[:, 2 * chunk_base:2 * (chunk_base + nch[p])])
                    nc.sync.dma_start(
                        wgx[:], w_in_[:, 2 * chunk_base:2 * (chunk_base + nch[p])])
                    for grp in range(ncalls // 8):
                        ssrc = spool.tile([128, 8, feat], dt.float32, tag="ssrc")
                        for gi in range(8):
                            cg = grp * 8 + gi
                            stg = spool.tile([128, GCALL // 128, feat],
                                             dt.float32, tag="stg")
                            nc.gpsimd.dma_gather(
                                out_ap=stg[:],
                                in_ap=table_aps[p],
                                idxs_ap=gix[:, cg * (GCALL // 16):
                                            (cg + 1) * (GCALL // 16)],
                                num_idxs=GCALL, num_idxs_reg=GCALL,
                                elem_size=feat,
                                queue_num=pool_dma_counter[0] % 4)
                            pool_dma_counter[0] += 1
                            ps = psum_pool.tile([128, feat], dt.float32,
                                                tag="partials")
                            for k in range(4):
                                ci = cg * 4 + k
                                for half in range(2):
                                    col = 2 * ci + half
                                    S = qpool.tile([128, MAXSEG], dt.float32,
                                                   tag="S")
                                    nc.vector.tensor_scalar(
                                        out=S[:], in0=iota_sb[:],
                                        scalar1=sgx[:, col:col + 1],
                                        scalar2=wgx[:, col:col + 1],
                                        op0=Alu.is_equal, op1=Alu.mult)
                                    nc.tensor.matmul(
                                        out=ps[32 * k:32 * (k + 1), :],
                                        lhsT=S[:], rhs=stg[:, 2 * k + half, :],
                                        start=(half == 0), stop=(half == 1),
                                        tile_position=(0, 32 * k))
                            nc.vector.tensor_copy(ssrc[:, gi, :], ps[:])
                        nc.gpsimd.dma_scatter_add(
                            out_ap=agg_dram[:],
                            in_ap=ssrc[:],
                            idxs_ap=sidx_sb[:, (chunk_base * MAXSEG + grp * GCALL) // 16:
                                            (chunk_base * MAXSEG + (grp + 1) * GCALL) // 16],
                            num_idxs=GCALL, num_idxs_reg=GCALL,
                            elem_size=feat,
                            queue_num=pool_dma_counter[0] % 4)
                        pool_dma_counter[0] += 1
                    chunk_base += nch[p]

            # ---------------- feature-major layer tail ----------------
            def emit_build_uT(hprev_nm_ap, agg_dram, feat, eps_col):
                """uT_dram[:feat] = transpose(eps*hprev + agg) per node tile."""
                ntile_full = SHARD // 128
                rem = SHARD - ntile_full * 128
                for tt in range(ntile_full + 1):
                    rows = 128 if tt < ntile_full else rem
                    hb = qpool.tile([128, feat], dt.float32, tag="hb")
                    ab = qpool.tile([128, feat], dt.float32, tag="ab")
                    nc.sync.dma_start(hb[:rows, :],
                                      hprev_nm_ap[128 * tt:128 * tt + rows, :])
                    nc.scalar.dma_start(ab[:rows, :],
                                        agg_dram[128 * tt:128 * tt + rows, :])
                    cmb = qpool.tile([128, feat], dt.float32, tag="cmb")
                    if rows < 128:
                        nc.gpsimd.memset(cmb[:], 0.0)
                    nc.vector.tensor_scalar(
                        out=cmb[:rows, :], in0=hb[:rows, :],
                        scalar1=eps_col, scalar2=None, op0=Alu.mult)
                    nc.vector.tensor_tensor(
                        out=cmb[:rows, :], in0=cmb[:rows, :], in1=ab[:rows, :],
                        op=Alu.add)
                    pt = psum_pool.tile([feat, 128], dt.float32, tag="tp")
                    nc.tensor.transpose(out=pt[:], in_=cmb[:, :feat],
                                        identity=ident[:])
                    us = qpool.tile([feat, 128], dt.float32, tag="us")
                    nc.vector.tensor_copy(us[:, :rows], pt[:, :rows])
                    nc.scalar.dma_start(
                        uT_dram[:feat, 128 * tt:128 * tt + rows], us[:, :rows])

            def emit_mlp_bn(feat, Wa_ap, Wb_ap, ba_ap, bb_ap, g_ap, be_ap,
                            hprevT_ap, out_hT_dram, out_nm_ap, tbls):
                # pass A: z2 = (relu(uT Wa + ba)) Wb + bb, accumulate stats
                sums = qpool.tile([64, 2], dt.float32, tag="sums")
                nc.gpsimd.memset(sums[:], 0.0)
                for g in range(NGRP):
                    c0 = 512 * g
                    c1 = min(c0 + 512, SHARD)
                    w = c1 - c0
                    ub = bpool.tile([feat, 512], dt.float32, tag="ub")
                    nc.sync.dma_start(ub[:, :w], uT_dram[:feat, c0:c1])
                    pz = psum_pool.tile([64, 512], dt.float32, tag="pz")
                    nc.tensor.matmul(out=pz[:, :w], lhsT=Wa_ap,
                                     rhs=ub[:feat, :w], start=True, stop=True)
                    zb = bpool.tile([64, 512], dt.float32, tag="zb")
                    nc.scalar.activation(zb[:, :w], pz[:, :w], Act.Relu,
                                         bias=ba_ap, scale=1.0)
                    pz2 = psum_pool.tile([64, 512], dt.float32, tag="pz2")
                    nc.tensor.matmul(out=pz2[:, :w], lhsT=Wb_ap,
                                     rhs=zb[:, :w], start=True, stop=True)
                    z2b = bpool.tile([64, 512], dt.float32, tag="z2b")
                    nc.scalar.activation(z2b[:, :w], pz2[:, :w], Act.Identity,
                                         bias=bb_ap, scale=1.0)
                    nc.scalar.dma_start(z2T_dram[:, c0:c1], z2b[:, :w])
                    # stats
                    part = qpool.tile([64, 2], dt.float32, tag="part")
                    sq = bpool.tile([64, 512], dt.float32, tag="sq")
                    nc.vector.tensor_tensor(out=sq[:, :w], in0=z2b[:, :w],
                                            in1=z2b[:, :w], op=Alu.mult)
                    nc.vector.tensor_reduce(part[:, 0:1], z2b[:, :w],
                                            axis=mybir.AxisListType.X, op=Alu.add)
                    nc.vector.tensor_reduce(part[:, 1:2], sq[:, :w],
                                            axis=mybir.AxisListType.X, op=Alu.add)
                    nc.vector.tensor_tensor(out=sums[:], in0=sums[:],
                                            in1=part[:], op=Alu.add)
                nc.sync.dma_start(red_io[:], sums[:])
                nc.gpsimd.collective_compute(
                    "AllReduce", Alu.add, replica_groups=groups,
                    ins=[red_io.opt()], outs=[red_oo.opt()])
                gsum = qpool.tile([64, 2], dt.float32, tag="gsum")
                nc.sync.dma_start(gsum[:], red_oo[:])
                stat = qpool.tile([64, 4], dt.float32, tag="stat")
                # mean | Ex2
                nc.vector.tensor_scalar(out=stat[:, 0:2], in0=gsum[:],
                                        scalar1=1.0 / N, scalar2=None,
                                        op0=Alu.mult)
                # var = Ex2 - mean^2
                nc.vector.tensor_tensor(out=stat[:, 2:3], in0=stat[:, 0:1],
                                        in1=stat[:, 0:1], op=Alu.mult)
                nc.vector.tensor_tensor(out=stat[:, 2:3], in0=stat[:, 1:2],
                                        in1=stat[:, 2:3], op=Alu.subtract)
                # rstd = 1/sqrt(var + eps)
                std = qpool.tile([64, 1], dt.float32, tag="std")
                nc.scalar.activation(std[:], stat[:, 2:3], Act.Sqrt,
                                     bias=epsb[:], scale=1.0)
                rstd = qpool.tile([64, 1], dt.float32, tag="rstd")
                nc.vector.reciprocal(rstd[:], std[:])
                scsh = qpool.tile([64, 2], dt.float32, tag="scsh")
                nc.vector.tensor_tensor(out=scsh[:, 0:1], in0=g_ap,
                                        in1=rstd[:], op=Alu.mult)
                nc.vector.tensor_tensor(out=scsh[:, 1:2], in0=stat[:, 0:1],
                                        in1=scsh[:, 0:1], op=Alu.mult)
                nc.vector.tensor_tensor(out=scsh[:, 1:2], in0=be_ap,
                                        in1=scsh[:, 1:2], op=Alu.subtract)
                # pass B: h = relu(z2*scale + shift) (+ residual), transpose out
                ntile_full = SHARD // 128
                for g in range(NGRP):
                    c0 = 512 * g
                    c1 = min(c0 + 512, SHARD)
                    w = c1 - c0
                    z2b = bpool.tile([64, 512], dt.float32, tag="z2rb")
                    nc.sync.dma_start(z2b[:, :w], z2T_dram[:, c0:c1])
                    hb = bpool.tile([64, 512], dt.float32, tag="hnb")
                    nc.scalar.activation(hb[:, :w], z2b[:, :w], Act.Relu,
                                         bias=scsh[:, 1:2], scale=scsh[:, 0:1])
                    if hprevT_ap is not None:
                        hp = bpool.tile([64, 512], dt.float32, tag="hpb")
                        nc.scalar.dma_start(hp[:, :w], hprevT_ap[:, c0:c1])
                        nc.vector.tensor_tensor(out=hb[:, :w], in0=hb[:, :w],
                                                in1=hp[:, :w], op=Alu.add)
                    if out_hT_dram is not None:
                        nc.scalar.dma_start(out_hT_dram[:, c0:c1], hb[:, :w])
                    # transpose to node-major and store
                    for q in range((w + 127) // 128):
                        r0 = c0 + 128 * q
                        rows = min(128, SHARD - r0)
                        tsrc = qpool.tile([128, 128], dt.float32, tag="tsrc")
                        nc.gpsimd.memset(tsrc[:], 0.0)
                        nc.vector.tensor_copy(
                            tsrc[:64, :rows], hb[:, 128 * q:128 * q + rows])
                        ptf = psum_pool.tile([128, 128], dt.float32, tag="tp")
                        nc.tensor.transpose(out=ptf[:], in_=tsrc[:],
                                            identity=ident[:])
                        hseg = qpool.tile([128, 64], dt.float32, tag="hseg")
                        nc.vector.tensor_copy(hseg[:rows, :], ptf[:rows, :64])
                        nc.sync.dma_start(out_nm_ap[r0:r0 + rows, :],
                                          hseg[:rows, :])
                if tbls is not None:
                    for j in range(4):
                        nc.gpsimd.collective_compute(
                            "AllGather", Alu.bypass, replica_groups=groups,
                            ins=[out_nm_ap[SLICE * j:SLICE * (j + 1), :].opt()],
                            outs=[tbls[j].opt()])

            # ================= layer 1 =================
            emit_aggregation(nch1, F,
                             [x_in[WIN1 * p:WIN1 * (p + 1), :] for p in range(4)],
                             gidx1_in, seg1_in, w1_in, sidx1_sb, agg[0])
            emit_build_uT(x_in[0:SHARD, :], agg[0], F, 1.0 + eps1)
            emit_mlp_bn(F, w1a_sb[:], w1b_sb[:], vecs_sb[:, 0:1],
                        vecs_sb[:, 9:10], vecs_sb[:, 1:2], vecs_sb[:, 2:3],
                        None, hT_dram[0], shard_nm[0][:], tables[0])

            # ================= layer 2 =================
            emit_aggregation(nch2, H,
                             [tables[0][p][:] for p in range(4)],
                             gidx2_in, seg2_in, w2_in, sidx2_sb, agg[1])
            emit_build_uT(shard_nm[0][:], agg[1], H, 1.0 + float(epss[0]))
            emit_mlp_bn(H, wsa_sb[:, 0:64], wsb_sb[:, 0:64], vecs_sb[:, 3:4],
                        vecs_sb[:, 10:11], vecs_sb[:, 4:5], vecs_sb[:, 5:6],
                        hT_dram[0][:], hT_dram[1], shard_nm[1][:], tables[1])

            # ================= layer 3 =================
            emit_aggregation(nch2, H,
                             [tables[1][p][:] for p in range(4)],
                             gidx2_in, seg2_in, w2_in, sidx2_sb, agg[2])
            emit_build_uT(shard_nm[1][:], agg[2], H, 1.0 + float(epss[1]))
            emit_mlp_bn(H, wsa_sb[:, 64:128], wsb_sb[:, 64:128],
                        vecs_sb[:, 6:7], vecs_sb[:, 11:12], vecs_sb[:, 7:8],
                        vecs_sb[:, 8:9], hT_dram[1][:], hT_dram[2],
                        out_t[:], None)

    nc.compile()

    # Align each SWDGE instruction's ring (queue_num) with the DMASW
    # semaphore lane Tile assigned it (sem k -> queue k%4), so every
    # semaphore is only ever updated from one queue.
    import re as _re
    for fn in nc.m.functions:
        for bb in fn.blocks:
            for ins in bb.instructions:
                if type(ins).__name__ in ("InstDMAGatherAnt",
                                          "InstDMAScatterAddAnt"):
                    si = ins.sync_info
                    qn = None
                    if si is not None:
                        for upd in si.on_update:
                            m = _re.match(r"DMASW(\d+)_", upd.ant_name or "")
                            if m:
                                qn = int(m.group(1)) % 4
                                break
                    if qn is not None:
                        ins.queue_num = qn
    return nc


# --------------------------------------------------------------------------
# entry point
# --------------------------------------------------------------------------

def kernel(**inputs):
    from concourse.bass_utils import run_bass_kernel_spmd

    x = np.ascontiguousarray(np.asarray(inputs["x"], np.float32))
    edge_index = np.asarray(inputs["edge_index"])
    edge_weight = np.asarray(inputs["edge_weight"], np.float32)
    eps1 = float(np.asarray(inputs["eps1"]))
    epss = np.asarray(inputs["epss"], np.float32)

    metas, nch1, nch2 = _preprocess(edge_index, edge_weight)
    nc = _build_program(nch1, nch2, eps1, epss)

    vecs = np.zeros((64, 16), np.float32)
    vecs[:, 0] = np.asarray(inputs["b1a"], np.float32)
    vecs[:, 1] = np.asarray(inputs["g1"], np.float32)
    vecs[:, 2] = np.asarray(inputs["be1"], np.float32)
    vecs[:, 3] = np.asarray(inputs["bsa"], np.float32)[0]
    vecs[:, 4] = np.asarray(inputs["gs"], np.float32)[0]
    vecs[:, 5] = np.asarray(inputs["bes"], np.float32)[0]
    vecs[:, 6] = np.asarray(inputs["bsa"], np.float32)[1]
    vecs[:, 7] = np.asarray(inputs["gs"], np.float32)[1]
    vecs[:, 8] = np.asarray(inputs["bes"], np.float32)[1]
    vecs[:, 9] = np.asarray(inputs["b1b"], np.float32)
    vecs[:, 10] = np.asarray(inputs["bsb"], np.float32)[0]
    vecs[:, 11] = np.asarray(inputs["bsb"], np.float32)[1]
    vecs[:, 12] = 1.0 + eps1
    vecs[:, 13] = 1.0 + float(epss[0])
    vecs[:, 14] = 1.0 + float(epss[1])
    iota = np.tile(np.arange(MAXSEG, dtype=np.float32), (128, 1))

    in_maps = []
    for c in range(N_CORES):
        m1, m2 = metas[c]
        in_maps.append({
            "x_in": np.roll(x, -SHARD * c, axis=0),
            "gidx1": m1["gidx"], "seg1": m1["segid"], "w1": m1["wcol"],
            "sidx1": m1["sidx"],
            "gidx2": m2["gidx"], "seg2": m2["segid"], "w2": m2["wcol"],
            "sidx2": m2["sidx"],
            "iota": iota,
            "w1a": np.asarray(inputs["W1a"], np.float32),
            "w1b": np.asarray(inputs["W1b"], np.float32),
            "wsa": np.asarray(inputs["Wsa"], np.float32),
            "wsb": np.asarray(inputs["Wsb"], np.float32),
            "vecs": vecs,
        })

    import os
    global LAST_EXEC_NS
    trace = bool(os.environ.get("KERNEL_TRACE"))
    try:
        res = run_bass_kernel_spmd(nc, in_maps, core_ids=list(range(N_CORES)),
                                   trace=trace)
    except Exception:
        if not trace:
            raise
        res = run_bass_kernel_spmd(nc, in_maps, core_ids=list(range(N_CORES)))
    LAST_EXEC_NS = getattr(res, "exec_time_ns", None)
    out = np.concatenate([res.results[c]["out"] for c in range(N_CORES)], 0)
    return np.ascontiguousarray(out, dtype=np.float32)
